# revision 1
# baseline (speedup 1.0000x reference)
"""CARAFE forward on 8 TRN2 NeuronCores.

Problem: features (8,128,64,64) f32, masks (8,25,128,128) f32
         -> out (8,128,128,128) f32, KERNEL_SIZE=5, GROUP=1, SCALE=2.

Sharding: pure data-parallel, one batch sample per core.

Formulation (banded matmul, i-pairs stacked along K):
  out[c, 2h+p, 2w+q] = sum_{i,j} f[c, h+i-2, w+j-2] * m[i*5+j, 2h+p, 2w+q]
For fixed (h, i) this is a matmul over x = w+j-2 (K=64):
  PSUM[c, col(p,w,q)] += sum_x f[c, r, x] * S(h,i)[x, col],  r = h+i-2
  S(h,i)[w+j-2, p*128+2w+q] = m[5i+j, 2h+p, 2w+q]  (banded; entries whose
  feature column is zero-padded are dropped).
Two consecutive i's share K=128 by stacking features of rows r and r+1:
  FT2[x, h, c] = f[c, h, x], FT2[64+x, h, c] = f[c, h+1, x]
and stacking the two S blocks. Per h: 3 matmuls (i-pairs {0,1}, {2,3}, {4}),
K=128, N=256, accumulated into a 256-column slice of an 8-h PSUM mega-tile
(2048 cols, 4 banks). One DVE evacuation + one fully-contiguous output DMA
per 8 h's. All layout prep is host-side numpy (no FLOPs); all arithmetic
runs on the PE in float16 (~3e-4 rel err vs the fp32 reference).
"""

import numpy as np

N_CORES = 8
C, H, W = 128, 64, 64
K5 = 5
PAD = 2
KX = W                    # 64; K=128 after i-pair stacking
NCOL = 256                # (p, wo) output columns per low-res row h
NG = 3                    # i-groups per h: {0,1}, {2,3}, {4}
HB = 8                    # h rows per PSUM mega-tile / evacuation block
S_CHUNK = 16              # h rows per streamed S chunk

_compiled = {}


def _build_program(n_reps: int = 1):
    """Build the SPMD bass program; n_reps>1 unrolls the whole body for
    wall-clock timing contrasts (no hardware loop - branches are slow)."""
    import concourse.bacc as bacc
    import concourse.mybir as mybir
    import concourse.tile as tile

    dt16 = mybir.dt.float16
    nc = bacc.Bacc("TRN2", target_bir_lowering=False, debug=False,
                   num_devices=N_CORES)

    ft2 = nc.dram_tensor("ft2", [2 * KX, H, C], dt16, kind="ExternalInput")
    s2 = nc.dram_tensor("s2", [2 * KX, H, NG, NCOL], dt16,
                        kind="ExternalInput")
    out = nc.dram_tensor("out", [C, 2 * H, 2 * W], mybir.dt.float32,
                         kind="ExternalOutput")

    def h_matmuls(h):
        """(lhsT_h_index, k_size, group) triples for output row h."""
        mm = []
        if h == 1:
            mm.append((0, KX, 0))          # i=1 alone: lhsT top = f[:,0]
        elif h >= 2:
            mm.append((h - 2, 2 * KX, 0))  # i={0,1}: rows h-2, h-1
        mm.append((h, 2 * KX if h < H - 1 else KX, 1))  # i={2,3}: rows h, h+1
        if h + 2 < H:
            mm.append((h + 2, 2 * KX, 2))  # i=4: row h+2 (bottom half zero S)
        return mm

    def body(sb, ss, ps, ob):
        ft_t = sb.tile([2 * KX, H, C], dt16, tag="ft")
        nc.sync.dma_start(ft_t[:], ft2[:])
        for h0 in range(0, H, S_CHUNK):
            s_t = ss.tile([2 * KX, S_CHUNK, NG, NCOL], dt16, tag="s")
            nc.sync.dma_start(s_t[:], s2[:, h0:h0 + S_CHUNK, :, :])
            for b0 in range(h0, h0 + S_CHUNK, HB):
                acc = ps.tile([C, HB * NCOL], mybir.dt.float32)
                for hl in range(HB):
                    h = b0 + hl
                    mms = h_matmuls(h)
                    o_sl = acc[:, hl * NCOL:(hl + 1) * NCOL]
                    for n_i, (hw, ks, g) in enumerate(mms):
                        nc.tensor.matmul(
                            o_sl, ft_t[0:ks, hw, :],
                            s_t[0:ks, h - h0, g, :],
                            start=(n_i == 0), stop=(n_i == len(mms) - 1))
                o = ob.tile([C, HB * NCOL], mybir.dt.float32, tag="o")
                nc.vector.tensor_copy(o[:], acc[:])
                nc.sync.dma_start(
                    out[:, 2 * b0:2 * (b0 + HB), :],
                    o[:].rearrange("c (hp w) -> c hp w", w=2 * W))

    with tile.TileContext(nc) as tc:
        with (
            tc.tile_pool(name="sb", bufs=1) as sb,
            tc.tile_pool(name="ss", bufs=2) as ss,
            tc.tile_pool(name="ps", bufs=2, space="PSUM") as ps,
            tc.tile_pool(name="ob", bufs=3) as ob,
        ):
            for _ in range(n_reps):
                body(sb, ss, ps, ob)

    nc.compile()
    return nc


def _band(masks_n, i):
    """S(h,i) banded matrix for all h: [KX, H, 2, W, 2] from one sample's
    masks [25, 2H, 2W]; S[w+j-2, h, p, w, q] = m[5i+j, 2h+p, 2w+q]."""
    m = masks_n.reshape(K5, K5, H, 2, W, 2)  # [i, j, h, p, w, q]
    s = np.zeros((KX, H, 2, W, 2), dtype=np.float16)
    for j in range(K5):
        wlo = max(0, PAD - j)
        whi = min(W, W + PAD - j)
        wi = np.arange(wlo, whi)
        # dims (w, h, p, q) on both sides
        s[wi + j - PAD, :, :, wi, :] = m[i, j, :, :, wlo:whi].transpose(
            2, 0, 1, 3)
    return s


def _prep_inputs(features: np.ndarray, masks: np.ndarray):
    """Host-side layout prep (no FLOPs): stacked FT2 and grouped banded S2."""
    n = features.shape[0]
    ftw = features.transpose(0, 3, 2, 1).astype(np.float16)  # [n, w, h, c]
    ft2 = np.zeros((n, 2 * KX, H, C), dtype=np.float16)
    ft2[:, :KX] = ftw
    ft2[:, KX:, :H - 1] = ftw[:, :, 1:]      # row h+1; zero at h = H-1

    s2 = np.zeros((n, 2 * KX, H, NG, NCOL), dtype=np.float16)
    for smp in range(n):
        bands = [_band(masks[smp], i).reshape(KX, H, NCOL) for i in range(K5)]
        # group 0: i=0 (top, rows h-2 valid h>=2), i=1 (bottom, valid h>=1)
        s2[smp, :KX, 2:, 0] = bands[0][:, 2:]
        s2[smp, KX:, 2:, 0] = bands[1][:, 2:]
        s2[smp, :KX, 1, 0] = bands[1][:, 1]   # h=1 special: i=1 on top half
        # group 1: i=2 (top, always), i=3 (bottom, valid h <= H-2)
        s2[smp, :KX, :, 1] = bands[2]
        s2[smp, KX:, :H - 1, 1] = bands[3][:, :H - 1]
        # group 2: i=4 (top, valid h <= H-3); bottom stays zero
        s2[smp, :KX, :H - 2, 2] = bands[4][:, :H - 2]
    return ft2, s2


def kernel(features: np.ndarray, masks: np.ndarray) -> np.ndarray:
    from concourse.bass_utils import run_bass_kernel_spmd

    if 1 not in _compiled:
        _compiled[1] = _build_program(1)
    nc = _compiled[1]

    ft2, s2 = _prep_inputs(np.asarray(features, dtype=np.float32),
                           np.asarray(masks, dtype=np.float32))
    in_maps = [{"ft2": ft2[i], "s2": s2[i]} for i in range(N_CORES)]
    res = run_bass_kernel_spmd(nc, in_maps, list(range(N_CORES)))
    return np.stack([res.results[i]["out"] for i in range(N_CORES)], axis=0)



# revision 5
# speedup vs baseline: 650.4663x; 650.4663x over previous
"""CARAFE forward on 8 TRN2 NeuronCores.

Problem: features (8,128,64,64) f32, masks (8,25,128,128) f32
         -> out (8,128,128,128) f32, KERNEL_SIZE=5, GROUP=1, SCALE=2.

Sharding: pure data-parallel, one batch sample per core.

Formulation (banded matmul, i-pairs stacked along K):
  out[c, 2h+p, 2w+q] = sum_{i,j} f[c, h+i-2, w+j-2] * m[i*5+j, 2h+p, 2w+q]
For fixed (h, i) this is a matmul over x = w+j-2 (K=64):
  PSUM[c, col(p,w,q)] += sum_x f[c, r, x] * S(h,i)[x, col],  r = h+i-2
  S(h,i)[w+j-2, p*128+2w+q] = m[5i+j, 2h+p, 2w+q]  (banded; entries whose
  feature column is zero-padded are dropped).
Two consecutive i's share K=128 by stacking features of rows r and r+1:
  FT2[x, h, c] = f[c, h, x], FT2[64+x, h, c] = f[c, h+1, x]
Per h, 3 matmuls accumulate one 256-col PSUM slice:
  g0: i-pair {0,1}, K=128 (rows h-2, h-1), rhs slice of s01
  g1: i-pair {2,3}, K=128 (rows h,  h+1),  rhs slice of s01
  g2: i={4} alone,  K=64  (row  h+2),      rhs slice of s2b  (K=64 tensor -
      storing this band at K=64 instead of a zero-padded K=128 cuts 2.1MB/core
      of HBM traffic)
8 h's share one PSUM mega-tile (2048 cols, 4 banks); evacuation alternates
between DVE and ACT (fp32->fp16 cast on copy) into an fp16 staging tile, and
one contiguous fp16 output DMA per 8 h's. The fp16->fp32 widening of the
output happens on the host (rel err of fp16 storage ~5e-4, well within
tolerance); this halves output HBM traffic. All layout prep is host-side
numpy (no FLOPs); PE arithmetic runs in fp16 (~3e-4 rel err vs fp32 ref).

Per-core HBM traffic: ft2 2.10MB + s01 8.39MB + s2b 2.10MB + out 4.19MB
= 16.78MB (vs 23.1MB for the v1 layout)."""

import numpy as np

N_CORES = 8
C, H, W = 128, 64, 64
K5 = 5
PAD = 2
KX = W                    # 64; K=128 after i-pair stacking
NCOL = 256                # (p, wo) output columns per low-res row h
HB = 8                    # h rows per PSUM mega-tile / evacuation block
S_CHUNK = 16              # h rows per streamed S chunk

_compiled = {}


def _build_program_cached(n_reps: int = 1):
    if n_reps not in _compiled:
        _compiled[n_reps] = _build_program(n_reps)
    return _compiled[n_reps]


def _build_program(n_reps: int = 1):
    """Build the SPMD bass program; n_reps>1 unrolls the whole body for
    wall-clock timing contrasts (dispatch overhead cancels in the slope)."""
    import concourse.bacc as bacc
    import concourse.mybir as mybir
    import concourse.tile as tile

    dt16 = mybir.dt.float16
    nc = bacc.Bacc("TRN2", target_bir_lowering=False, debug=False,
                   num_devices=N_CORES)

    ft2 = nc.dram_tensor("ft2", [2 * KX, H, C], dt16, kind="ExternalInput")
    s01 = nc.dram_tensor("s01", [2 * KX, H, 2, NCOL], dt16,
                         kind="ExternalInput")
    s2b = nc.dram_tensor("s2b", [KX, H, NCOL], dt16, kind="ExternalInput")
    out = nc.dram_tensor("out", [C, 2 * H, 2 * W], dt16,
                         kind="ExternalOutput")

    def h_matmuls(h):
        """(lhsT_h_index, k_size, group) triples for output row h.
        group 0/1 read s01; group 2 reads s2b (K=64)."""
        mm = []
        if h == 1:
            mm.append((0, KX, 0))          # i=1 alone: lhsT top = f[:,0]
        elif h >= 2:
            mm.append((h - 2, 2 * KX, 0))  # i={0,1}: rows h-2, h-1
        mm.append((h, 2 * KX if h < H - 1 else KX, 1))  # i={2,3}: rows h, h+1
        if h + 2 < H:
            mm.append((h + 2, KX, 2))      # i=4: row h+2, K=64
        return mm

    def body(sb, ss, s2p, ps, ob):
        ft_t = sb.tile([2 * KX, H, C], dt16, tag="ft")
        nc.sync.dma_start(ft_t[:], ft2[:])
        blk = 0
        for h0 in range(0, H, S_CHUNK):
            s_t = ss.tile([2 * KX, S_CHUNK, 2, NCOL], dt16, tag="s")
            nc.sync.dma_start(s_t[:], s01[:, h0:h0 + S_CHUNK, :, :])
            s2_t = s2p.tile([KX, S_CHUNK, NCOL], dt16, tag="s2")
            nc.sync.dma_start(s2_t[:], s2b[:, h0:h0 + S_CHUNK, :])
            for b0 in range(h0, h0 + S_CHUNK, HB):
                acc = ps.tile([C, HB * NCOL], mybir.dt.float32)
                for hl in range(HB):
                    h = b0 + hl
                    mms = h_matmuls(h)
                    o_sl = acc[:, hl * NCOL:(hl + 1) * NCOL]
                    for n_i, (hw, ks, g) in enumerate(mms):
                        rhs = (s_t[0:ks, h - h0, g, :] if g < 2
                               else s2_t[0:ks, h - h0, :])
                        nc.tensor.matmul(
                            o_sl, ft_t[0:ks, hw, :], rhs,
                            start=(n_i == 0), stop=(n_i == len(mms) - 1))
                o = ob.tile([C, HB * NCOL], dt16, tag="o")
                if blk % 2 == 0:
                    nc.vector.tensor_copy(o[:], acc[:])
                else:
                    nc.scalar.activation(o[:], acc[:],
                                         mybir.ActivationFunctionType.Copy)
                blk += 1
                nc.sync.dma_start(
                    out[:, 2 * b0:2 * (b0 + HB), :],
                    o[:].rearrange("c (hp w) -> c hp w", w=2 * W))

    with tile.TileContext(nc) as tc:
        with (
            tc.tile_pool(name="sb", bufs=1) as sb,
            tc.tile_pool(name="ss", bufs=2) as ss,
            tc.tile_pool(name="s2p", bufs=2) as s2p,
            tc.tile_pool(name="ps", bufs=2, space="PSUM") as ps,
            tc.tile_pool(name="ob", bufs=3) as ob,
        ):
            for _ in range(n_reps):
                body(sb, ss, s2p, ps, ob)

    nc.compile()
    return nc


def _band(masks_n, i):
    """S(h,i) banded matrix for all h: [KX, H, 2, W, 2] from one sample's
    masks [25, 2H, 2W]; S[w+j-2, h, p, w, q] = m[5i+j, 2h+p, 2w+q]."""
    m = masks_n.reshape(K5, K5, H, 2, W, 2)  # [i, j, h, p, w, q]
    s = np.zeros((KX, H, 2, W, 2), dtype=np.float16)
    for j in range(K5):
        wlo = max(0, PAD - j)
        whi = min(W, W + PAD - j)
        wi = np.arange(wlo, whi)
        # dims (w, h, p, q) on both sides
        s[wi + j - PAD, :, :, wi, :] = m[i, j, :, :, wlo:whi].transpose(
            2, 0, 1, 3)
    return s


def _prep_inputs(features: np.ndarray, masks: np.ndarray):
    """Host-side layout prep (no FLOPs): stacked FT2, banded s01 (i-groups
    {0,1} and {2,3}) and K=64 banded s2b (i=4)."""
    n = features.shape[0]
    ftw = features.transpose(0, 3, 2, 1).astype(np.float16)  # [n, w, h, c]
    ft2 = np.zeros((n, 2 * KX, H, C), dtype=np.float16)
    ft2[:, :KX] = ftw
    ft2[:, KX:, :H - 1] = ftw[:, :, 1:]      # row h+1; zero at h = H-1

    s01 = np.zeros((n, 2 * KX, H, 2, NCOL), dtype=np.float16)
    s2b = np.zeros((n, KX, H, NCOL), dtype=np.float16)
    for smp in range(n):
        bands = [_band(masks[smp], i).reshape(KX, H, NCOL) for i in range(K5)]
        # group 0: i=0 (top, rows h-2 valid h>=2), i=1 (bottom, valid h>=1)
        s01[smp, :KX, 2:, 0] = bands[0][:, 2:]
        s01[smp, KX:, 2:, 0] = bands[1][:, 2:]
        s01[smp, :KX, 1, 0] = bands[1][:, 1]   # h=1 special: i=1 on top half
        # group 1: i=2 (top, always), i=3 (bottom, valid h <= H-2)
        s01[smp, :KX, :, 1] = bands[2]
        s01[smp, KX:, :H - 1, 1] = bands[3][:, :H - 1]
        # i=4 band, K=64, valid h <= H-3 (rows h=62,63 stay zero, never read)
        s2b[smp, :, :H - 2] = bands[4][:, :H - 2]
    return ft2, s01, s2b


def _in_maps(features: np.ndarray, masks: np.ndarray):
    ft2, s01, s2b = _prep_inputs(features, masks)
    return [{"ft2": ft2[i], "s01": s01[i], "s2b": s2b[i]}
            for i in range(N_CORES)]


def kernel(features: np.ndarray, masks: np.ndarray) -> np.ndarray:
    from concourse.bass_utils import run_bass_kernel_spmd

    nc = _build_program_cached(1)
    in_maps = _in_maps(np.asarray(features, dtype=np.float32),
                       np.asarray(masks, dtype=np.float32))
    res = run_bass_kernel_spmd(nc, in_maps, list(range(N_CORES)))
    out16 = np.stack([res.results[i]["out"] for i in range(N_CORES)], axis=0)
    return out16.astype(np.float32)


# revision 10
# speedup vs baseline: 712.4250x; 1.0953x over previous
"""CARAFE forward on 8 TRN2 NeuronCores.

Problem: features (8,128,64,64) f32, masks (8,25,128,128) f32
         -> out (8,128,128,128) f32, KERNEL_SIZE=5, GROUP=1, SCALE=2.

Sharding: pure data-parallel, one batch sample per core.

Formulation (banded matmul, i-pairs stacked along K):
  out[c, 2h+p, 2w+q] = sum_{i,j} f[c, h+i-2, w+j-2] * m[i*5+j, 2h+p, 2w+q]
For fixed (h, i) this is a matmul over x = w+j-2 (K=64):
  PSUM[c, col(p,w,q)] += sum_x f[c, r, x] * S(h,i)[x, col],  r = h+i-2
  S(h,i)[w+j-2, p*128+2w+q] = m[5i+j, 2h+p, 2w+q]  (banded; entries whose
  feature column is zero-padded are dropped).
Two consecutive i's share K=128 by stacking features of rows r and r+1:
  FT2[x, h, c] = f[c, h, x], FT2[64+x, h, c] = f[c, h+1, x]
Per h, 3 matmuls accumulate one 256-col PSUM slice:
  g0: i-pair {0,1}, K=128 (rows h-2, h-1), rhs slice of s01
  g1: i-pair {2,3}, K=128 (rows h,  h+1),  rhs slice of s01
  g2: i={4} alone,  K=64  (row  h+2),      rhs slice of s2b  (K=64 tensor -
      storing this band at K=64 instead of a zero-padded K=128 cuts 2.1MB/core
      of HBM traffic)
8 h's share one PSUM mega-tile (2048 cols, 4 banks); evacuation alternates
between DVE and ACT (fp32->fp16 cast on copy) into an fp16 staging tile, and
one contiguous fp16 output DMA per 8 h's. The fp16->fp32 widening of the
output happens on the host (rel err of fp16 storage ~5e-4, well within
tolerance); this halves output HBM traffic. All layout prep is host-side
numpy (no FLOPs); PE arithmetic runs in fp16 (~3e-4 rel err vs fp32 ref).

Per-core HBM traffic: ft2 2.10MB + s01 8.39MB + s2b 2.10MB + out 4.19MB
= 16.78MB (vs 23.1MB for the v1 layout)."""

import numpy as np

N_CORES = 8
C, H, W = 128, 64, 64
K5 = 5
PAD = 2
KX = W                    # 64; K=128 after i-pair stacking
NCOL = 256                # (p, wo) output columns per low-res row h
HB = 8                    # h rows per PSUM mega-tile / evacuation block
S_CHUNK = 32              # h rows per streamed S chunk (4.2MB s01 transfers)
OB_H = 32                 # h rows per staged output DMA (2.1MB transfers)

_compiled = {}


def _build_program_cached(n_reps: int = 1):
    if n_reps not in _compiled:
        _compiled[n_reps] = _build_program(n_reps)
    return _compiled[n_reps]


def _build_program(n_reps: int = 1, s_chunk: int = S_CHUNK, ob_h: int = OB_H):
    """Build the SPMD bass program; n_reps>1 unrolls the whole body for
    wall-clock timing contrasts (dispatch overhead cancels in the slope)."""
    import concourse.bacc as bacc
    import concourse.mybir as mybir
    import concourse.tile as tile

    dt16 = mybir.dt.float16
    nc = bacc.Bacc("TRN2", target_bir_lowering=False, debug=False,
                   num_devices=N_CORES)

    ft2 = nc.dram_tensor("ft2", [2 * KX, H, C], dt16, kind="ExternalInput")
    s01 = nc.dram_tensor("s01", [2 * KX, H, 2, NCOL], dt16,
                         kind="ExternalInput")
    s2b = nc.dram_tensor("s2b", [KX, H, NCOL], dt16, kind="ExternalInput")
    out = nc.dram_tensor("out", [C, 2 * H, 2 * W], dt16,
                         kind="ExternalOutput")

    def h_matmuls(h):
        """(lhsT_h_index, k_size, group) triples for output row h.
        group 0/1 read s01; group 2 reads s2b (K=64)."""
        mm = []
        if h == 1:
            mm.append((0, KX, 0))          # i=1 alone: lhsT top = f[:,0]
        elif h >= 2:
            mm.append((h - 2, 2 * KX, 0))  # i={0,1}: rows h-2, h-1
        mm.append((h, 2 * KX if h < H - 1 else KX, 1))  # i={2,3}: rows h, h+1
        if h + 2 < H:
            mm.append((h + 2, KX, 2))      # i=4: row h+2, K=64
        return mm

    def body(sb, ss, s2p, ps, ob):
        ft_t = sb.tile([2 * KX, H, C], dt16, tag="ft")
        nc.sync.dma_start(ft_t[:], ft2[:])
        blk = 0
        for h0 in range(0, H, s_chunk):
            s_t = ss.tile([2 * KX, s_chunk, 2, NCOL], dt16, tag="s")
            nc.sync.dma_start(s_t[:], s01[:, h0:h0 + s_chunk, :, :])
            s2_t = s2p.tile([KX, s_chunk, NCOL], dt16, tag="s2")
            nc.sync.dma_start(s2_t[:], s2b[:, h0:h0 + s_chunk, :])
            for o0 in range(h0, h0 + s_chunk, ob_h):
                o = ob.tile([C, ob_h * NCOL], dt16, tag="o")
                for b0 in range(o0, o0 + ob_h, HB):
                    acc = ps.tile([C, HB * NCOL], mybir.dt.float32)
                    for hl in range(HB):
                        h = b0 + hl
                        mms = h_matmuls(h)
                        o_sl = acc[:, hl * NCOL:(hl + 1) * NCOL]
                        for n_i, (hw, ks, g) in enumerate(mms):
                            rhs = (s_t[0:ks, h - h0, g, :] if g < 2
                                   else s2_t[0:ks, h - h0, :])
                            nc.tensor.matmul(
                                o_sl, ft_t[0:ks, hw, :], rhs,
                                start=(n_i == 0), stop=(n_i == len(mms) - 1))
                    o_dst = o[:, (b0 - o0) * NCOL:(b0 - o0 + HB) * NCOL]
                    if blk % 2 == 0:
                        nc.vector.tensor_copy(o_dst, acc[:])
                    else:
                        nc.scalar.activation(o_dst, acc[:],
                                             mybir.ActivationFunctionType.Copy)
                    blk += 1
                nc.sync.dma_start(
                    out[:, 2 * o0:2 * (o0 + ob_h), :],
                    o[:].rearrange("c (hp w) -> c hp w", w=2 * W))

    with tile.TileContext(nc) as tc:
        with (
            tc.tile_pool(name="sb", bufs=1) as sb,
            tc.tile_pool(name="ss", bufs=2) as ss,
            tc.tile_pool(name="s2p", bufs=2) as s2p,
            tc.tile_pool(name="ps", bufs=2, space="PSUM") as ps,
            tc.tile_pool(name="ob", bufs=3) as ob,
        ):
            for _ in range(n_reps):
                body(sb, ss, s2p, ps, ob)

    nc.compile()
    return nc


def _band(masks_n, i):
    """S(h,i) banded matrix for all h: [KX, H, 2, W, 2] from one sample's
    masks [25, 2H, 2W]; S[w+j-2, h, p, w, q] = m[5i+j, 2h+p, 2w+q]."""
    m = masks_n.reshape(K5, K5, H, 2, W, 2)  # [i, j, h, p, w, q]
    s = np.zeros((KX, H, 2, W, 2), dtype=np.float16)
    for j in range(K5):
        wlo = max(0, PAD - j)
        whi = min(W, W + PAD - j)
        wi = np.arange(wlo, whi)
        # dims (w, h, p, q) on both sides
        s[wi + j - PAD, :, :, wi, :] = m[i, j, :, :, wlo:whi].transpose(
            2, 0, 1, 3)
    return s


def _prep_inputs(features: np.ndarray, masks: np.ndarray):
    """Host-side layout prep (no FLOPs): stacked FT2, banded s01 (i-groups
    {0,1} and {2,3}) and K=64 banded s2b (i=4)."""
    n = features.shape[0]
    ftw = features.transpose(0, 3, 2, 1).astype(np.float16)  # [n, w, h, c]
    ft2 = np.zeros((n, 2 * KX, H, C), dtype=np.float16)
    ft2[:, :KX] = ftw
    ft2[:, KX:, :H - 1] = ftw[:, :, 1:]      # row h+1; zero at h = H-1

    s01 = np.zeros((n, 2 * KX, H, 2, NCOL), dtype=np.float16)
    s2b = np.zeros((n, KX, H, NCOL), dtype=np.float16)
    for smp in range(n):
        bands = [_band(masks[smp], i).reshape(KX, H, NCOL) for i in range(K5)]
        # group 0: i=0 (top, rows h-2 valid h>=2), i=1 (bottom, valid h>=1)
        s01[smp, :KX, 2:, 0] = bands[0][:, 2:]
        s01[smp, KX:, 2:, 0] = bands[1][:, 2:]
        s01[smp, :KX, 1, 0] = bands[1][:, 1]   # h=1 special: i=1 on top half
        # group 1: i=2 (top, always), i=3 (bottom, valid h <= H-2)
        s01[smp, :KX, :, 1] = bands[2]
        s01[smp, KX:, :H - 1, 1] = bands[3][:, :H - 1]
        # i=4 band, K=64, valid h <= H-3 (rows h=62,63 stay zero, never read)
        s2b[smp, :, :H - 2] = bands[4][:, :H - 2]
    return ft2, s01, s2b


def _in_maps(features: np.ndarray, masks: np.ndarray):
    ft2, s01, s2b = _prep_inputs(features, masks)
    return [{"ft2": ft2[i], "s01": s01[i], "s2b": s2b[i]}
            for i in range(N_CORES)]


def kernel(features: np.ndarray, masks: np.ndarray) -> np.ndarray:
    from concourse.bass_utils import run_bass_kernel_spmd

    nc = _build_program_cached(1)
    in_maps = _in_maps(np.asarray(features, dtype=np.float32),
                       np.asarray(masks, dtype=np.float32))
    res = run_bass_kernel_spmd(nc, in_maps, list(range(N_CORES)))
    out16 = np.stack([res.results[i]["out"] for i in range(N_CORES)], axis=0)
    return out16.astype(np.float32)


# revision 11
# speedup vs baseline: 796.4584x; 1.1180x over previous
"""CARAFE forward on 8 TRN2 NeuronCores.

Problem: features (8,128,64,64) f32, masks (8,25,128,128) f32
         -> out (8,128,128,128) f32, KERNEL_SIZE=5, GROUP=1, SCALE=2.

Sharding: pure data-parallel, one batch sample per core.

Formulation (banded matmul, i-pairs stacked along K):
  out[c, 2h+p, 2w+q] = sum_{i,j} f[c, h+i-2, w+j-2] * m[i*5+j, 2h+p, 2w+q]
For fixed (h, i) this is a matmul over x = w+j-2 (K=64):
  PSUM[c, col(p,w,q)] += sum_x f[c, r, x] * S(h,i)[x, col],  r = h+i-2
  S(h,i)[w+j-2, p*128+2w+q] = m[5i+j, 2h+p, 2w+q]  (banded; entries whose
  feature column is zero-padded are dropped).
Two consecutive i's share K=128 by stacking features of rows r and r+1:
  FT2[x, h, c] = f[c, h, x], FT2[64+x, h, c] = f[c, h+1, x]
Per h, 3 matmuls accumulate one 256-col PSUM slice:
  g0: i-pair {0,1}, K=128 (rows h-2, h-1), rhs slice of s01
  g1: i-pair {2,3}, K=128 (rows h,  h+1),  rhs slice of s01
  g2: i={4} alone,  K=64  (row  h+2),      rhs slice of s2b  (K=64 tensor -
      storing this band at K=64 instead of a zero-padded K=128 cuts 2.1MB/core
      of HBM traffic)
8 h's share one PSUM mega-tile (2048 cols, 4 banks); evacuation alternates
between DVE and ACT (fp32->fp16 cast on copy) into an fp16 staging tile, and
one contiguous fp16 output DMA per 8 h's. The fp16->fp32 widening of the
output happens on the host (rel err of fp16 storage ~5e-4, well within
tolerance); this halves output HBM traffic. All layout prep is host-side
numpy (no FLOPs); PE arithmetic runs in fp16 (~3e-4 rel err vs fp32 ref).

Per-core HBM traffic: ft2 2.10MB + s01 8.39MB + s2b 2.10MB + out 4.19MB
= 16.78MB (vs 23.1MB for the v1 layout)."""

import numpy as np

N_CORES = 8
C, H, W = 128, 64, 64
K5 = 5
PAD = 2
KX = W                    # 64; K=128 after i-pair stacking
NCOL = 256                # (p, wo) output columns per low-res row h
HB = 8                    # h rows per PSUM mega-tile / evacuation block
S_CHUNK = 32              # h rows per streamed S chunk (4.2MB s01 transfers)
OB_H = 32                 # h rows per staged output DMA (2.1MB transfers)

_compiled = {}


def _build_program_cached(n_reps: int = 1):
    if n_reps not in _compiled:
        _compiled[n_reps] = _build_program(n_reps)
    return _compiled[n_reps]


def _build_program(n_reps: int = 1, s_chunk: int = S_CHUNK, ob_h: int = OB_H):
    """Build the SPMD bass program; n_reps>1 unrolls the whole body for
    wall-clock timing contrasts (dispatch overhead cancels in the slope)."""
    import concourse.bacc as bacc
    import concourse.mybir as mybir
    import concourse.tile as tile

    dt16 = mybir.dt.float16
    nc = bacc.Bacc("TRN2", target_bir_lowering=False, debug=False,
                   num_devices=N_CORES)

    ft2 = nc.dram_tensor("ft2", [2 * KX, H, C], dt16, kind="ExternalInput")
    s01 = nc.dram_tensor("s01", [2 * KX, H, 2, NCOL], dt16,
                         kind="ExternalInput")
    # i=4 band packed to 128 partitions: even h on partitions 0:64, odd h on
    # 64:128 (full-rate DMA). For odd h the K=64 stationary (feature row h+2)
    # is read from ft2's bottom half at index h+1, so lhsT and rhs share
    # base partition 64 (a legal PE tile position).
    s2b = nc.dram_tensor("s2b", [2 * KX, H // 2, NCOL], dt16,
                         kind="ExternalInput")
    out = nc.dram_tensor("out", [C, 2 * H, 2 * W], dt16,
                         kind="ExternalOutput")

    def h_matmuls(h):
        """(lhsT_h_index, k_size, group) triples for output row h.
        group 0/1 read s01; group 2 reads s2b (K=64)."""
        mm = []
        if h == 1:
            mm.append((0, KX, 0))          # i=1 alone: lhsT top = f[:,0]
        elif h >= 2:
            mm.append((h - 2, 2 * KX, 0))  # i={0,1}: rows h-2, h-1
        mm.append((h, 2 * KX if h < H - 1 else KX, 1))  # i={2,3}: rows h, h+1
        if h + 2 < H:
            mm.append((h + 2, KX, 2))      # i=4: row h+2, K=64
        return mm

    def body(sb, ss, s2p, ps, ob):
        ft_t = sb.tile([2 * KX, H, C], dt16, tag="ft")
        nc.sync.dma_start(ft_t[:], ft2[:])
        blk = 0
        for h0 in range(0, H, s_chunk):
            s_t = ss.tile([2 * KX, s_chunk, 2, NCOL], dt16, tag="s")
            nc.sync.dma_start(s_t[:], s01[:, h0:h0 + s_chunk, :, :])
            s2_t = s2p.tile([KX, s_chunk, NCOL], dt16, tag="s2")
            nc.sync.dma_start(s2_t[:], s2b[:, h0:h0 + s_chunk, :])
            for o0 in range(h0, h0 + s_chunk, ob_h):
                o = ob.tile([C, ob_h * NCOL], dt16, tag="o")
                for b0 in range(o0, o0 + ob_h, HB):
                    acc = ps.tile([C, HB * NCOL], mybir.dt.float32)
                    for hl in range(HB):
                        h = b0 + hl
                        mms = h_matmuls(h)
                        o_sl = acc[:, hl * NCOL:(hl + 1) * NCOL]
                        for n_i, (hw, ks, g) in enumerate(mms):
                            rhs = (s_t[0:ks, h - h0, g, :] if g < 2
                                   else s2_t[0:ks, h - h0, :])
                            nc.tensor.matmul(
                                o_sl, ft_t[0:ks, hw, :], rhs,
                                start=(n_i == 0), stop=(n_i == len(mms) - 1))
                    o_dst = o[:, (b0 - o0) * NCOL:(b0 - o0 + HB) * NCOL]
                    if blk % 2 == 0:
                        nc.vector.tensor_copy(o_dst, acc[:])
                    else:
                        nc.scalar.activation(o_dst, acc[:],
                                             mybir.ActivationFunctionType.Copy)
                    blk += 1
                nc.sync.dma_start(
                    out[:, 2 * o0:2 * (o0 + ob_h), :],
                    o[:].rearrange("c (hp w) -> c hp w", w=2 * W))

    with tile.TileContext(nc) as tc:
        with (
            tc.tile_pool(name="sb", bufs=1) as sb,
            tc.tile_pool(name="ss", bufs=2) as ss,
            tc.tile_pool(name="s2p", bufs=2) as s2p,
            tc.tile_pool(name="ps", bufs=2, space="PSUM") as ps,
            tc.tile_pool(name="ob", bufs=3) as ob,
        ):
            for _ in range(n_reps):
                body(sb, ss, s2p, ps, ob)

    nc.compile()
    return nc


def _band(masks_n, i):
    """S(h,i) banded matrix for all h: [KX, H, 2, W, 2] from one sample's
    masks [25, 2H, 2W]; S[w+j-2, h, p, w, q] = m[5i+j, 2h+p, 2w+q]."""
    m = masks_n.reshape(K5, K5, H, 2, W, 2)  # [i, j, h, p, w, q]
    s = np.zeros((KX, H, 2, W, 2), dtype=np.float16)
    for j in range(K5):
        wlo = max(0, PAD - j)
        whi = min(W, W + PAD - j)
        wi = np.arange(wlo, whi)
        # dims (w, h, p, q) on both sides
        s[wi + j - PAD, :, :, wi, :] = m[i, j, :, :, wlo:whi].transpose(
            2, 0, 1, 3)
    return s


def _prep_inputs(features: np.ndarray, masks: np.ndarray):
    """Host-side layout prep (no FLOPs): stacked FT2, banded s01 (i-groups
    {0,1} and {2,3}) and K=64 banded s2b (i=4)."""
    n = features.shape[0]
    ftw = features.transpose(0, 3, 2, 1).astype(np.float16)  # [n, w, h, c]
    ft2 = np.zeros((n, 2 * KX, H, C), dtype=np.float16)
    ft2[:, :KX] = ftw
    ft2[:, KX:, :H - 1] = ftw[:, :, 1:]      # row h+1; zero at h = H-1

    s01 = np.zeros((n, 2 * KX, H, 2, NCOL), dtype=np.float16)
    s2b = np.zeros((n, KX, H, NCOL), dtype=np.float16)
    for smp in range(n):
        bands = [_band(masks[smp], i).reshape(KX, H, NCOL) for i in range(K5)]
        # group 0: i=0 (top, rows h-2 valid h>=2), i=1 (bottom, valid h>=1)
        s01[smp, :KX, 2:, 0] = bands[0][:, 2:]
        s01[smp, KX:, 2:, 0] = bands[1][:, 2:]
        s01[smp, :KX, 1, 0] = bands[1][:, 1]   # h=1 special: i=1 on top half
        # group 1: i=2 (top, always), i=3 (bottom, valid h <= H-2)
        s01[smp, :KX, :, 1] = bands[2]
        s01[smp, KX:, :H - 1, 1] = bands[3][:, :H - 1]
        # i=4 band, K=64, valid h <= H-3 (rows h=62,63 stay zero, never read)
        s2b[smp, :, :H - 2] = bands[4][:, :H - 2]
    return ft2, s01, s2b


def _in_maps(features: np.ndarray, masks: np.ndarray):
    ft2, s01, s2b = _prep_inputs(features, masks)
    return [{"ft2": ft2[i], "s01": s01[i], "s2b": s2b[i]}
            for i in range(N_CORES)]


def kernel(features: np.ndarray, masks: np.ndarray) -> np.ndarray:
    from concourse.bass_utils import run_bass_kernel_spmd

    nc = _build_program_cached(1)
    in_maps = _in_maps(np.asarray(features, dtype=np.float32),
                       np.asarray(masks, dtype=np.float32))
    res = run_bass_kernel_spmd(nc, in_maps, list(range(N_CORES)))
    out16 = np.stack([res.results[i]["out"] for i in range(N_CORES)], axis=0)
    return out16.astype(np.float32)


# revision 15
# speedup vs baseline: 1769.5250x; 2.2217x over previous
"""CARAFE forward on 8 TRN2 NeuronCores.

Problem: features (8,128,64,64) f32, masks (8,25,128,128) f32
         -> out (8,128,128,128) f32, KERNEL_SIZE=5, GROUP=1, SCALE=2.

Sharding: pure data-parallel, one batch sample per core.

Formulation (banded matmul, i-pairs stacked along K):
  out[c, 2h+p, 2w+q] = sum_{i,j} f[c, h+i-2, w+j-2] * m[i*5+j, 2h+p, 2w+q]
For fixed (h, i) this is a matmul over x = w+j-2 (K=64):
  PSUM[c, col(p,w,q)] += sum_x f[c, r, x] * S(h,i)[x, col],  r = h+i-2
  S(h,i)[w+j-2, p*128+2w+q] = m[5i+j, 2h+p, 2w+q]  (banded; entries whose
  feature column is zero-padded are dropped).
Two consecutive i's share K=128 by stacking features of rows r and r+1:
  FT2[x, h, c] = f[c, h, x], FT2[64+x, h, c] = f[c, h+1, x]
Per h, 3 matmuls accumulate one 256-col PSUM slice:
  g0: i-pair {0,1}, K=128 (rows h-2, h-1), rhs slice of s01
  g1: i-pair {2,3}, K=128 (rows h,  h+1),  rhs slice of s01
  g2: i={4} alone,  K=64  (row  h+2),      rhs slice of s2b
s2b stores the i=4 band at K=64 (vs zero-padded K=128: -2.1MB/core HBM) and
packs even h on partitions 0:64 / odd h on 64:128 so its DMA uses all 128
partition ports; for odd h the stationary (feature row h+2) is read from
ft2's bottom half at index h+1, putting lhsT and rhs both at base partition
64 (legal PE tile position (64, 0)).
8 h's share one PSUM mega-tile (2048 cols, 4 banks); evacuation alternates
between DVE and ACT (fp32->fp16 cast on copy) into an fp16 staging tile
covering 32 h's, DMA'd out as one contiguous 2.1MB fp16 transfer. The
fp16->fp32 widening of the output happens on the host (fp16 storage adds
~2e-4 rel err, well within tolerance); this halves output HBM traffic. All
layout prep is host-side numpy (no FLOPs); PE arithmetic runs in fp16
(~3e-4 rel err vs the fp32 reference).

Per-core HBM traffic: ft2 2.10MB + s01 8.39MB + s2b 2.10MB + out 4.19MB
= 16.78MB (vs 23.1MB for the v1 layout). Measured steady-state body time
~44-49us (wall-clock slope between unroll factors, device-resident inputs),
~87%% of the 39.5us DMA-byte roofline at 425GB/s/core."""

import numpy as np

N_CORES = 8
C, H, W = 128, 64, 64
K5 = 5
PAD = 2
KX = W                    # 64; K=128 after i-pair stacking
NCOL = 256                # (p, wo) output columns per low-res row h
HB = 8                    # h rows per PSUM mega-tile / evacuation block
S_CHUNK = 32              # h rows per streamed S chunk (4.2MB s01 transfers)
OB_H = 32                 # h rows per staged output DMA (2.1MB transfers)

_compiled = {}


def _build_program_cached(n_reps: int = 1):
    if n_reps not in _compiled:
        _compiled[n_reps] = _build_program(n_reps)
    return _compiled[n_reps]


def _build_program(n_reps: int = 1, s_chunk: int = S_CHUNK, ob_h: int = OB_H):
    """Build the SPMD bass program; n_reps>1 unrolls the whole body for
    wall-clock timing contrasts (dispatch overhead cancels in the slope)."""
    import concourse.bacc as bacc
    import concourse.mybir as mybir
    import concourse.tile as tile

    dt16 = mybir.dt.float16
    nc = bacc.Bacc("TRN2", target_bir_lowering=False, debug=False,
                   num_devices=N_CORES)

    ft2 = nc.dram_tensor("ft2", [2 * KX, H, C], dt16, kind="ExternalInput")
    s01 = nc.dram_tensor("s01", [2 * KX, H, 2, NCOL], dt16,
                         kind="ExternalInput")
    # i=4 band packed to 128 partitions: even h on partitions 0:64, odd h on
    # 64:128 (full-rate DMA). For odd h the K=64 stationary (feature row h+2)
    # is read from ft2's bottom half at index h+1, so lhsT and rhs share
    # base partition 64 (a legal PE tile position).
    s2b = nc.dram_tensor("s2b", [2 * KX, H // 2, NCOL], dt16,
                         kind="ExternalInput")
    out = nc.dram_tensor("out", [C, 2 * H, 2 * W], dt16,
                         kind="ExternalOutput")

    def h_matmuls(h):
        """(lhsT_h_index, k_size, group) triples for output row h.
        group 0/1 read s01; group 2 reads s2b (K=64)."""
        mm = []
        if h == 1:
            mm.append((0, KX, 0))          # i=1 alone: lhsT top = f[:,0]
        elif h >= 2:
            mm.append((h - 2, 2 * KX, 0))  # i={0,1}: rows h-2, h-1
        mm.append((h, 2 * KX if h < H - 1 else KX, 1))  # i={2,3}: rows h, h+1
        if h + 2 < H:
            mm.append((h + 2, KX, 2))      # i=4: row h+2, K=64
        return mm

    def body(sb, ss, s2p, ps, ob):
        ft_t = sb.tile([2 * KX, H, C], dt16, tag="ft")
        nc.sync.dma_start(ft_t[:], ft2[:])
        blk = 0
        for h0 in range(0, H, s_chunk):
            s_t = ss.tile([2 * KX, s_chunk, 2, NCOL], dt16, tag="s")
            nc.sync.dma_start(s_t[:], s01[:, h0:h0 + s_chunk, :, :])
            s2_t = s2p.tile([2 * KX, s_chunk // 2, NCOL], dt16, tag="s2")
            nc.sync.dma_start(s2_t[:], s2b[:, h0 // 2:(h0 + s_chunk) // 2, :])
            for o0 in range(h0, h0 + s_chunk, ob_h):
                o = ob.tile([C, ob_h * NCOL], dt16, tag="o")
                for b0 in range(o0, o0 + ob_h, HB):
                    acc = ps.tile([C, HB * NCOL], mybir.dt.float32)
                    for hl in range(HB):
                        h = b0 + hl
                        mms = h_matmuls(h)
                        o_sl = acc[:, hl * NCOL:(hl + 1) * NCOL]
                        for n_i, (hw, ks, g) in enumerate(mms):
                            if g < 2:
                                lhs = ft_t[0:ks, hw, :]
                                rhs = s_t[0:ks, h - h0, g, :]
                            elif h % 2 == 0:
                                # even h: g2 on partitions 0:64 (row h+2 at
                                # ft2 top half, index h+2)
                                lhs = ft_t[0:KX, hw, :]
                                rhs = s2_t[0:KX, (h - h0) // 2, :]
                            else:
                                # odd h: g2 on partitions 64:128 (row h+2 is
                                # ft2's bottom half at index h+1)
                                lhs = ft_t[KX:2 * KX, hw - 1, :]
                                rhs = s2_t[KX:2 * KX, (h - h0) // 2, :]
                            nc.tensor.matmul(
                                o_sl, lhs, rhs,
                                start=(n_i == 0), stop=(n_i == len(mms) - 1))
                    o_dst = o[:, (b0 - o0) * NCOL:(b0 - o0 + HB) * NCOL]
                    if blk % 2 == 0:
                        nc.vector.tensor_copy(o_dst, acc[:])
                    else:
                        nc.scalar.activation(o_dst, acc[:],
                                             mybir.ActivationFunctionType.Copy)
                    blk += 1
                nc.sync.dma_start(
                    out[:, 2 * o0:2 * (o0 + ob_h), :],
                    o[:].rearrange("c (hp w) -> c hp w", w=2 * W))

    with tile.TileContext(nc) as tc:
        with (
            tc.tile_pool(name="sb", bufs=1) as sb,
            tc.tile_pool(name="ss", bufs=2) as ss,
            tc.tile_pool(name="s2p", bufs=2) as s2p,
            tc.tile_pool(name="ps", bufs=2, space="PSUM") as ps,
            tc.tile_pool(name="ob", bufs=3) as ob,
        ):
            for _ in range(n_reps):
                body(sb, ss, s2p, ps, ob)

    nc.compile()
    return nc


def _band(masks_n, i):
    """S(h,i) banded matrix for all h: [KX, H, 2, W, 2] from one sample's
    masks [25, 2H, 2W]; S[w+j-2, h, p, w, q] = m[5i+j, 2h+p, 2w+q]."""
    m = masks_n.reshape(K5, K5, H, 2, W, 2)  # [i, j, h, p, w, q]
    s = np.zeros((KX, H, 2, W, 2), dtype=np.float16)
    for j in range(K5):
        wlo = max(0, PAD - j)
        whi = min(W, W + PAD - j)
        wi = np.arange(wlo, whi)
        # dims (w, h, p, q) on both sides
        s[wi + j - PAD, :, :, wi, :] = m[i, j, :, :, wlo:whi].transpose(
            2, 0, 1, 3)
    return s


def _prep_inputs(features: np.ndarray, masks: np.ndarray):
    """Host-side layout prep (no FLOPs): stacked FT2, banded s01 (i-groups
    {0,1} and {2,3}) and K=64 banded s2b (i=4)."""
    n = features.shape[0]
    ftw = features.transpose(0, 3, 2, 1).astype(np.float16)  # [n, w, h, c]
    ft2 = np.zeros((n, 2 * KX, H, C), dtype=np.float16)
    ft2[:, :KX] = ftw
    ft2[:, KX:, :H - 1] = ftw[:, :, 1:]      # row h+1; zero at h = H-1

    s01 = np.zeros((n, 2 * KX, H, 2, NCOL), dtype=np.float16)
    s2b = np.zeros((n, 2 * KX, H // 2, NCOL), dtype=np.float16)
    for smp in range(n):
        bands = [_band(masks[smp], i).reshape(KX, H, NCOL) for i in range(K5)]
        # group 0: i=0 (top, rows h-2 valid h>=2), i=1 (bottom, valid h>=1)
        s01[smp, :KX, 2:, 0] = bands[0][:, 2:]
        s01[smp, KX:, 2:, 0] = bands[1][:, 2:]
        s01[smp, :KX, 1, 0] = bands[1][:, 1]   # h=1 special: i=1 on top half
        # group 1: i=2 (top, always), i=3 (bottom, valid h <= H-2)
        s01[smp, :KX, :, 1] = bands[2]
        s01[smp, KX:, :H - 1, 1] = bands[3][:, :H - 1]
        # i=4 band (valid h <= H-3; h=62,63 stay zero, never read), packed
        # to 128 partitions: even h on 0:64, odd h on 64:128
        s2b[smp, :KX] = bands[4][:, 0::2].astype(np.float16)
        s2b[smp, KX:] = bands[4][:, 1::2].astype(np.float16)
    return ft2, s01, s2b


def _in_maps(features: np.ndarray, masks: np.ndarray):
    ft2, s01, s2b = _prep_inputs(features, masks)
    return [{"ft2": ft2[i], "s01": s01[i], "s2b": s2b[i]}
            for i in range(N_CORES)]


def kernel(features: np.ndarray, masks: np.ndarray) -> np.ndarray:
    from concourse.bass_utils import run_bass_kernel_spmd

    nc = _build_program_cached(1)
    in_maps = _in_maps(np.asarray(features, dtype=np.float32),
                       np.asarray(masks, dtype=np.float32))
    res = run_bass_kernel_spmd(nc, in_maps, list(range(N_CORES)))
    out16 = np.stack([res.results[i]["out"] for i in range(N_CORES)], axis=0)
    return out16.astype(np.float32)
